# revision 16
# baseline (speedup 1.0000x reference)
"""Trainium2 Bass kernel for nn_DiffHistogram (Gaussian soft-binned histogram).

Computes, for x of shape [B=8, C=8, H=256, W=256] and 32 bin centers:
    out[b, c*32+k, 0, 0] = sum_{h,w} (ER/RATIO) * exp(-(clip(x)-c_k)^2 / (2*sigma^2))

Sharding: data-parallel over batch B across 8 NeuronCores; each core handles
one sample [C, H*W] and computes its full [C, 32] pooled histogram.

Per-core layout: SBUF tile [128, 4096] with partition p = (c*16 + g):
channel c in 0..7, pixel-group g in 0..15, 4096 pixels along free dim.
Per bin k (3-engine pipeline, raw Bass with manual semaphores):
  DVE : D_k = x_bf16 - c_k                (tensor_scalar, bf16, 4x mode)
  ACT : E_k = Derivative_Erf(sqrt(512)*D) (= 2/sqrt(pi) * exp(-512 d^2))
  DVE : acc[:, k] = sum_free(E_k)         (tensor_scalar accum_out, bf16 4x)
Final: PE matmul with block-ones lhsT (value folds ER/RATIO * sqrt(pi)/2)
reduces the 16 groups per channel -> psum [8, 32] -> SBUF -> DMA out.

The kernel is written in raw Bass (no TileContext): the Tile-emitted
program (attached sync_info on high-id virtual semaphores) does not
compile with this container's walrus build.
"""

import contextlib
import math
import os

import numpy as np

import concourse.bass as bass
import concourse.mybir as mybir
from concourse.bass_utils import run_bass_kernel_spmd

B = 8
C = 8
HW = 256 * 256          # 65536 pixels per channel
NBINS = 32
G = 128 // C            # 16 partition groups per channel
FREE = HW // G          # 4096 pixels per partition

ER = 1.0
RATIO = 2.5066
SIGMA = 1.0 / NBINS                        # (LAST-FIRST)/NBINS
INV_2SIG2 = 1.0 / (2.0 * SIGMA * SIGMA)    # 512.0
SQRT_INV_2SIG2 = math.sqrt(INV_2SIG2)      # 22.627417

# Derivative_Erf(t) = 2/sqrt(pi) * exp(-t^2); with t = sqrt(512)*d this is
# exp(-512 d^2) * 2/sqrt(pi). Fold the correction and ER/RATIO into the
# final reduction weights.
DERF_OUT_SCALE = (ER / RATIO) * (math.sqrt(math.pi) / 2.0)

ND = 3   # D (diff) buffers in flight
NE = 3   # E (weight) buffers in flight

_nc_cache: dict = {}
last_results = None  # BassKernelResults of the most recent run (for test.py)


def _build(bin_centers: np.ndarray) -> "bass.Bass":
    key = tuple(np.asarray(bin_centers, np.float64).tolist())
    if key in _nc_cache:
        return _nc_cache[key]

    f32 = mybir.dt.float32
    bf16 = mybir.dt.bfloat16
    alu = mybir.AluOpType
    act_fn = mybir.ActivationFunctionType

    lo = float(bin_centers[0])
    hi = float(bin_centers[-1])

    nc = bass.Bass("TRN2", target_bir_lowering=False, debug=False, num_devices=B)
    x_d = nc.dram_tensor("x", [C, HW], f32, kind="ExternalInput")
    w_d = nc.dram_tensor("w", [128, C], f32, kind="ExternalInput")
    out_d = nc.dram_tensor("out", [C, NBINS], f32, kind="ExternalOutput")

    with contextlib.ExitStack() as st:
        X = st.enter_context(nc.sbuf_tensor("X", [128, FREE], f32))
        Xcl = st.enter_context(nc.sbuf_tensor("Xcl", [128, FREE], f32))
        Xb = st.enter_context(nc.sbuf_tensor("Xb", [128, FREE], bf16))
        Ds = [
            st.enter_context(nc.sbuf_tensor(f"D{i}", [128, FREE], bf16))
            for i in range(ND)
        ]
        Es = [
            st.enter_context(nc.sbuf_tensor(f"E{i}", [128, FREE], bf16))
            for i in range(NE)
        ]
        Js = [
            st.enter_context(nc.sbuf_tensor(f"J{i}", [128, FREE], bf16))
            for i in range(2)
        ]
        acc = st.enter_context(nc.sbuf_tensor("acc", [128, NBINS], f32))
        ones = st.enter_context(nc.sbuf_tensor("ones", [128, C], f32))
        out_sb = st.enter_context(nc.sbuf_tensor("out_sb", [C, NBINS], f32))
        ps = st.enter_context(nc.psum_tensor("ps", [C, NBINS], f32))

        s_dma = st.enter_context(nc.semaphore("s_dma"))
        s_dmw = st.enter_context(nc.semaphore("s_dmw"))
        s_pool = st.enter_context(nc.semaphore("s_pool"))
        s_sub = st.enter_context(nc.semaphore("s_sub"))
        s_act = st.enter_context(nc.semaphore("s_act"))
        s_acc = st.enter_context(nc.semaphore("s_acc"))
        s_pe = st.enter_context(nc.semaphore("s_pe"))
        s_out = st.enter_context(nc.semaphore("s_out"))

        block = st.enter_context(nc.Block())

        @block.sync
        def _(sync):
            sync.dma_start(
                X.ap(), x_d.ap().rearrange("c (g j) -> (c g) j", g=G)
            ).then_inc(s_dma, 16)
            sync.dma_start(ones.ap(), w_d.ap()).then_inc(s_dmw, 16)
            sync.wait_ge(s_out, 1)
            sync.dma_start(out_d.ap(), out_sb.ap()).then_inc(s_dma, 16)

        @block.gpsimd
        def _(gp):
            gp.wait_ge(s_dma, 16)
            nc.gpsimd.tensor_scalar(Xcl.ap(), X.ap(), lo, None, op0=alu.max).then_inc(
                s_pool, 1
            )
            gp.wait_ge(s_pool, 1)  # Pool pipeline: order the two clip passes
            nc.gpsimd.tensor_scalar(Xb.ap(), Xcl.ap(), hi, None, op0=alu.min).then_inc(
                s_pool, 1
            )

        def emit_sub(k):
            ck = float(bin_centers[k])
            nc.vector.tensor_scalar(
                Ds[k % ND].ap(), Xb.ap(), ck, None, op0=alu.subtract
            ).then_inc(s_sub, 1)

        @block.vector
        def _(vector):
            vector.wait_ge(s_pool, 2)
            emit_sub(0)
            emit_sub(1)
            for k in range(NBINS):
                vector.wait_ge(s_act, k + 1)
                if k >= 2:
                    # J buffer reuse: DVE pipeline gives no same-engine WAW
                    # ordering; wait for the 2-back accum to retire.
                    vector.wait_ge(s_acc, k - 1)
                nc.vector.tensor_scalar(
                    Js[k % 2].ap(), Es[k % NE].ap(), 0.0, None,
                    op0=alu.bypass, op1=alu.add,
                    accum_out=acc.ap()[:, k : k + 1],
                ).then_inc(s_acc, 1)
                if k + 2 < NBINS:
                    emit_sub(k + 2)
            vector.wait_ge(s_pe, 1)
            nc.vector.tensor_copy(out_sb.ap(), ps.ap()).then_inc(s_out, 1)

        @block.scalar
        def _(scalar):
            for k in range(NBINS):
                scalar.wait_ge(s_sub, k + 1)
                if k >= NE:
                    scalar.wait_ge(s_acc, k - NE + 1)
                nc.scalar.activation(
                    Es[k % NE].ap(), Ds[k % ND].ap(),
                    act_fn.Derivative_Erf, scale=SQRT_INV_2SIG2,
                ).then_inc(s_act, 1)

        @block.tensor
        def _(tensor):
            tensor.wait_ge(s_dmw, 16)
            tensor.wait_ge(s_acc, NBINS)
            nc.tensor.matmul(
                ps.ap(), ones.ap(), acc.ap(), start=True, stop=True
            ).then_inc(s_pe, 1)

    _nc_cache[key] = nc
    return nc


def _block_ones() -> np.ndarray:
    w = np.zeros((128, C), np.float32)
    for c in range(C):
        w[c * G : (c + 1) * G, c] = DERF_OUT_SCALE
    return w


def kernel(x: np.ndarray, bin_centers: np.ndarray) -> np.ndarray:
    global last_results
    x = np.ascontiguousarray(np.asarray(x), dtype=np.float32)
    bc = np.asarray(bin_centers, dtype=np.float32)
    assert x.shape == (B, C, 256, 256), x.shape
    assert bc.shape == (NBINS,), bc.shape

    nc = _build(bc.astype(np.float64))

    w = _block_ones()
    in_maps = [{"x": x[b].reshape(C, HW), "w": w} for b in range(B)]
    res = run_bass_kernel_spmd(nc, in_maps, list(range(B)))
    last_results = res
    outs = [np.asarray(res.results[b]["out"], np.float32) for b in range(B)]
    return np.stack(outs).reshape(B, C * NBINS, 1, 1)


# revision 18
# speedup vs baseline: 4693.9193x; 4693.9193x over previous
"""Trainium2 Bass kernel for nn_DiffHistogram (Gaussian soft-binned histogram).

Computes, for x of shape [B=8, C=8, H=256, W=256] and 32 bin centers:
    out[b, c*32+k, 0, 0] = sum_{h,w} (ER/RATIO) * exp(-(clip(x)-c_k)^2 / (2*sigma^2))

Sharding: data-parallel over batch B across 8 NeuronCores; each core handles
one sample [C, H*W] and computes its full [C, 32] pooled histogram.

Per-core layout: SBUF tile [128, 4096] with partition p = (c*16 + g):
channel c in 0..7, pixel-group g in 0..15, 4096 pixels along free dim.
Per bin k (3-engine pipeline, raw Bass with manual semaphores):
  DVE : D_k = x_bf16 - c_k                (tensor_scalar, bf16, 4x mode)
  ACT : E_k = Derivative_Erf(sqrt(512)*D) (= 2/sqrt(pi) * exp(-512 d^2))
  DVE : acc[:, k] = sum_free(E_k)         (tensor_scalar accum_out, bf16 4x)
Final: PE matmul with block-ones lhsT (value folds ER/RATIO * sqrt(pi)/2)
reduces the 16 groups per channel -> psum [8, 32] -> SBUF -> DMA out.

The kernel is written in raw Bass (no TileContext): the Tile-emitted
program (attached sync_info on high-id virtual semaphores) does not
compile with this container's walrus build.
"""

import contextlib
import math
import os

import numpy as np

import concourse.bass as bass
import concourse.mybir as mybir
from concourse.bass_utils import run_bass_kernel_spmd

B = 8
C = 8
HW = 256 * 256          # 65536 pixels per channel
NBINS = 32
G = 128 // C            # 16 partition groups per channel
FREE = HW // G          # 4096 pixels per partition

ER = 1.0
RATIO = 2.5066
SIGMA = 1.0 / NBINS                        # (LAST-FIRST)/NBINS
INV_2SIG2 = 1.0 / (2.0 * SIGMA * SIGMA)    # 512.0
SQRT_INV_2SIG2 = math.sqrt(INV_2SIG2)      # 22.627417

# Derivative_Erf(t) = 2/sqrt(pi) * exp(-t^2); with t = sqrt(512)*d this is
# exp(-512 d^2) * 2/sqrt(pi). Fold the correction and ER/RATIO into the
# final reduction weights.
DERF_OUT_SCALE = (ER / RATIO) * (math.sqrt(math.pi) / 2.0)

ND = 3   # D (diff) buffers in flight
NE = 3   # E (weight) buffers in flight

_nc_cache: dict = {}
last_results = None  # BassKernelResults of the most recent run (for test.py)


def _build(bin_centers: np.ndarray, reps: int = 1) -> "bass.Bass":
    """Build the per-core program. reps > 1 repeats the full 32-bin body
    (recomputing acc each time) — used only for steady-state timing; the
    output is identical to reps=1."""
    key = (reps, tuple(np.asarray(bin_centers, np.float64).tolist()))
    if key in _nc_cache:
        return _nc_cache[key]
    T = reps * NBINS

    f32 = mybir.dt.float32
    bf16 = mybir.dt.bfloat16
    alu = mybir.AluOpType
    act_fn = mybir.ActivationFunctionType

    lo = float(bin_centers[0])
    hi = float(bin_centers[-1])

    nc = bass.Bass("TRN2", target_bir_lowering=False, debug=False, num_devices=B)
    x_d = nc.dram_tensor("x", [C, HW], f32, kind="ExternalInput")
    w_d = nc.dram_tensor("w", [128, C], f32, kind="ExternalInput")
    out_d = nc.dram_tensor("out", [C, NBINS], f32, kind="ExternalOutput")

    with contextlib.ExitStack() as st:
        X = st.enter_context(nc.sbuf_tensor("X", [128, FREE], f32))
        Xcl = st.enter_context(nc.sbuf_tensor("Xcl", [128, FREE], f32))
        Xb = st.enter_context(nc.sbuf_tensor("Xb", [128, FREE], bf16))
        Ds = [
            st.enter_context(nc.sbuf_tensor(f"D{i}", [128, FREE], bf16))
            for i in range(ND)
        ]
        Es = [
            st.enter_context(nc.sbuf_tensor(f"E{i}", [128, FREE], bf16))
            for i in range(NE)
        ]
        Js = [
            st.enter_context(nc.sbuf_tensor(f"J{i}", [128, FREE], bf16))
            for i in range(2)
        ]
        acc = st.enter_context(nc.sbuf_tensor("acc", [128, NBINS], f32))
        ones = st.enter_context(nc.sbuf_tensor("ones", [128, C], f32))
        out_sb = st.enter_context(nc.sbuf_tensor("out_sb", [C, NBINS], f32))
        ps = st.enter_context(nc.psum_tensor("ps", [C, NBINS], f32))

        s_dma = st.enter_context(nc.semaphore("s_dma"))
        s_dmw = st.enter_context(nc.semaphore("s_dmw"))
        s_pool = st.enter_context(nc.semaphore("s_pool"))
        s_sub = st.enter_context(nc.semaphore("s_sub"))
        s_act = st.enter_context(nc.semaphore("s_act"))
        s_acc = st.enter_context(nc.semaphore("s_acc"))
        s_pe = st.enter_context(nc.semaphore("s_pe"))
        s_out = st.enter_context(nc.semaphore("s_out"))

        block = st.enter_context(nc.Block())

        @block.sync
        def _(sync):
            sync.dma_start(
                X.ap(), x_d.ap().rearrange("c (g j) -> (c g) j", g=G)
            ).then_inc(s_dma, 16)
            sync.dma_start(ones.ap(), w_d.ap()).then_inc(s_dmw, 16)
            sync.wait_ge(s_out, 1)
            sync.dma_start(out_d.ap(), out_sb.ap()).then_inc(s_dma, 16)

        @block.gpsimd
        def _(gp):
            gp.wait_ge(s_dma, 16)
            nc.gpsimd.tensor_scalar(Xcl.ap(), X.ap(), lo, None, op0=alu.max).then_inc(
                s_pool, 1
            )
            gp.wait_ge(s_pool, 1)  # Pool pipeline: order the two clip passes
            nc.gpsimd.tensor_scalar(Xb.ap(), Xcl.ap(), hi, None, op0=alu.min).then_inc(
                s_pool, 1
            )

        def emit_sub(i):
            ck = float(bin_centers[i % NBINS])
            nc.vector.tensor_scalar(
                Ds[i % ND].ap(), Xb.ap(), ck, None, op0=alu.subtract
            ).then_inc(s_sub, 1)

        @block.vector
        def _(vector):
            vector.wait_ge(s_pool, 2)
            emit_sub(0)
            emit_sub(1)
            for i in range(T):
                vector.wait_ge(s_act, i + 1)
                if i >= 2:
                    # J buffer reuse: DVE pipeline gives no same-engine WAW
                    # ordering; wait for the 2-back accum to retire. (Also
                    # orders acc-column overwrites across reps.)
                    vector.wait_ge(s_acc, i - 1)
                nc.vector.tensor_scalar(
                    Js[i % 2].ap(), Es[i % NE].ap(), 0.0, None,
                    op0=alu.bypass, op1=alu.add,
                    accum_out=acc.ap()[:, (i % NBINS) : (i % NBINS) + 1],
                ).then_inc(s_acc, 1)
                if i + 2 < T:
                    emit_sub(i + 2)
            vector.wait_ge(s_pe, 1)
            nc.vector.tensor_copy(out_sb.ap(), ps.ap()).then_inc(s_out, 1)

        @block.scalar
        def _(scalar):
            for i in range(T):
                scalar.wait_ge(s_sub, i + 1)
                if i >= NE:
                    scalar.wait_ge(s_acc, i - NE + 1)
                nc.scalar.activation(
                    Es[i % NE].ap(), Ds[i % ND].ap(),
                    act_fn.Derivative_Erf, scale=SQRT_INV_2SIG2,
                ).then_inc(s_act, 1)

        @block.tensor
        def _(tensor):
            tensor.wait_ge(s_dmw, 16)
            tensor.wait_ge(s_acc, T)
            nc.tensor.matmul(
                ps.ap(), ones.ap(), acc.ap(), start=True, stop=True
            ).then_inc(s_pe, 1)

    _nc_cache[key] = nc
    return nc


def _block_ones() -> np.ndarray:
    w = np.zeros((128, C), np.float32)
    for c in range(C):
        w[c * G : (c + 1) * G, c] = DERF_OUT_SCALE
    return w


def kernel(x: np.ndarray, bin_centers: np.ndarray) -> np.ndarray:
    global last_results
    x = np.ascontiguousarray(np.asarray(x), dtype=np.float32)
    bc = np.asarray(bin_centers, dtype=np.float32)
    assert x.shape == (B, C, 256, 256), x.shape
    assert bc.shape == (NBINS,), bc.shape

    nc = _build(bc.astype(np.float64))

    w = _block_ones()
    in_maps = [{"x": x[b].reshape(C, HW), "w": w} for b in range(B)]
    res = run_bass_kernel_spmd(nc, in_maps, list(range(B)))
    last_results = res
    outs = [np.asarray(res.results[b]["out"], np.float32) for b in range(B)]
    return np.stack(outs).reshape(B, C * NBINS, 1, 1)
